# revision 7
# baseline (speedup 1.0000x reference)
import sys

sys.path.insert(0, "/opt/trn_rl_repo")
import numpy as np
from concourse import bass, bacc, tile, bass_utils

mybir = bass.mybir
F32 = mybir.dt.float32
BF16 = mybir.dt.bfloat16
NP_BF16 = np.dtype(mybir.dt.np(BF16))

N = 100000
E = 1600000
D = 128
NCORES = 8
NPC = N // NCORES
CHUNK = 512
SLOTS = 42  # attr slots per matmul group (42*3 = 126 contraction rows)


def _build(ncols, n_groups, kinds, reps=1):
    nc = bacc.Bacc(None, target_bir_lowering=False)
    xT_d = nc.dram_tensor("xT_d", [128, ncols], BF16, kind="ExternalInput")
    ap_d = [nc.dram_tensor(f"ap_d{g}", [126, ncols], BF16, kind="ExternalInput")
            for g in range(n_groups)]
    inv_d = nc.dram_tensor("inv_d", [1, ncols], BF16, kind="ExternalInput")
    w1a_d = nc.dram_tensor("w1a_d", [128, 128], F32, kind="ExternalInput")
    w1b_d = nc.dram_tensor("w1b_d", [128, 128], F32, kind="ExternalInput")
    w1c_d = nc.dram_tensor("w1c_d", [126, 128], BF16, kind="ExternalInput")
    w2_d = nc.dram_tensor("w2_d", [128, 128], BF16, kind="ExternalInput")
    b1_d = nc.dram_tensor("b1_d", [128, 1], F32, kind="ExternalInput")
    b2_d = nc.dram_tensor("b2_d", [128, 1], F32, kind="ExternalInput")
    ones_d = nc.dram_tensor("ones_d", [1, 128], BF16, kind="ExternalInput")
    out_d = nc.dram_tensor("out_d", [128, ncols], F32, kind="ExternalOutput")

    has_b = "B" in kinds
    relu = mybir.ActivationFunctionType.Relu
    ident = mybir.ActivationFunctionType.Identity

    with tile.TileContext(nc) as tc:
        with tc.tile_pool(name="const", bufs=1) as cp, \
             tc.tile_pool(name="work", bufs=3) as wp, \
             tc.tile_pool(name="ps", bufs=2, space="PSUM") as pp:
            xT = cp.tile([128, ncols], BF16, name="xT")
            ap = [cp.tile([126, ncols], BF16, name=f"ap{g}") for g in range(n_groups)]
            inv = cp.tile([1, ncols], BF16, name="inv")
            w1a_f = cp.tile([128, 128], F32, name="w1a_f")
            w1b_f = cp.tile([128, 128], F32, name="w1b_f")
            w1ab_f = cp.tile([128, 128], F32, name="w1ab_f")
            w1ab = cp.tile([128, 128], BF16, name="w1ab")
            w1c = cp.tile([126, 128], BF16, name="w1c")
            w2 = cp.tile([128, 128], BF16, name="w2")
            b1 = cp.tile([128, 1], F32, name="b1")
            b2 = cp.tile([128, 1], F32, name="b2")
            ones = cp.tile([1, 128], BF16, name="ones")
            for t, dt_ in [(xT, xT_d), (inv, inv_d), (w1a_f, w1a_d),
                           (w1b_f, w1b_d), (w1c, w1c_d), (w2, w2_d),
                           (b1, b1_d), (b2, b2_d), (ones, ones_d)]:
                nc.gpsimd.dma_start(t[:], dt_[:])
            for g in range(n_groups):
                nc.gpsimd.dma_start(ap[g][:], ap_d[g][:])

            nc.vector.tensor_tensor(out=w1ab_f[:], in0=w1a_f[:], in1=w1b_f[:],
                                    op=mybir.AluOpType.add)
            nc.vector.tensor_copy(w1ab[:], w1ab_f[:])
            if has_b:
                w1a_bf = cp.tile([128, 128], BF16, name="w1a_bf")
                nc.vector.tensor_copy(w1a_bf[:], w1a_f[:])

            for rep in range(reps):
              for i, kind in enumerate(kinds):
                lo, hi = i * CHUNK, (i + 1) * CHUNK
                P1 = pp.tile([128, CHUNK], F32, name="P1")
                if kind == "A":
                    Pinv = pp.tile([128, CHUNK], F32, name="Pinv")
                    nc.tensor.matmul(out=Pinv[:], lhsT=ones[:], rhs=inv[:, lo:hi],
                                     start=True, stop=True)
                    inv_b = wp.tile([128, CHUNK], BF16, name="inv_b")
                    nc.scalar.copy(inv_b[:], Pinv[:])
                    nc.tensor.matmul(out=P1[:], lhsT=w1ab[:], rhs=xT[:, lo:hi],
                                     start=True, stop=False)
                    for g in range(n_groups):
                        aps = wp.tile([126, CHUNK], BF16, name=f"aps{g}")
                        nc.vector.tensor_tensor(out=aps[:], in0=ap[g][:, lo:hi],
                                                in1=inv_b[:126, :],
                                                op=mybir.AluOpType.mult)
                        nc.tensor.matmul(out=P1[:], lhsT=w1c[:], rhs=aps[:],
                                         start=False, stop=(g == n_groups - 1))
                else:
                    nc.tensor.matmul(out=P1[:], lhsT=w1a_bf[:], rhs=xT[:, lo:hi],
                                     start=True, stop=True)
                h = wp.tile([128, CHUNK], BF16, name="h")
                nc.scalar.activation(out=h[:], in_=P1[:], func=relu, bias=b1[:])
                Pout = pp.tile([128, CHUNK], F32, name="Pout")
                nc.tensor.matmul(out=Pout[:], lhsT=w2[:], rhs=h[:],
                                 start=True, stop=True)
                ostage = wp.tile([128, CHUNK], F32, name="ostage")
                nc.scalar.activation(out=ostage[:], in_=Pout[:], func=ident,
                                     bias=b2[:])
                nc.gpsimd.dma_start(out_d[:, lo:hi], ostage[:])
    nc.compile()
    names = {
        "xT": xT_d.name, "ap": [d.name for d in ap_d], "inv": inv_d.name,
        "w1a": w1a_d.name, "w1b": w1b_d.name, "w1c": w1c_d.name,
        "w2": w2_d.name, "b1": b1_d.name, "b2": b2_d.name,
        "ones": ones_d.name, "out": out_d.name,
    }
    return nc, names


def _prepare(x, edge_index, edge_attr, W1, b1, W2, b2):
    x = np.asarray(x, np.float32)
    attr = np.asarray(edge_attr, np.float32)
    src = np.asarray(edge_index)[1].astype(np.int64)
    W1 = np.asarray(W1, np.float32)
    b1 = np.asarray(b1, np.float32)
    W2 = np.asarray(W2, np.float32)
    b2 = np.asarray(b2, np.float32)

    cnt = np.bincount(src, minlength=N)
    order = np.argsort(src, kind="stable")
    src_s = src[order]
    attr_s = attr[order]
    rowptr = np.zeros(N + 1, np.int64)
    rowptr[1:] = np.cumsum(cnt)
    occ = np.arange(E, dtype=np.int64) - rowptr[src_s]
    maxdeg = int(cnt.max())
    n_groups = max(1, -(-maxdeg // SLOTS))

    # per-core column assignment: A-region (cnt>0) then B-region (cnt==0)
    col_of = np.zeros(N, np.int64)
    a_lists, b_lists = [], []
    for c in range(NCORES):
        nodes = np.arange(c * NPC, (c + 1) * NPC)
        amask = cnt[nodes] > 0
        a_lists.append(nodes[amask])
        b_lists.append(nodes[~amask])
    ca = max(-(-len(a) // CHUNK) for a in a_lists)
    cb = max(-(-len(b) // CHUNK) for b in b_lists)
    ncols = CHUNK * (ca + cb)
    kinds = ["A"] * ca + ["B"] * cb
    for c in range(NCORES):
        col_of[a_lists[c]] = np.arange(len(a_lists[c]))
        col_of[b_lists[c]] = ca * CHUNK + np.arange(len(b_lists[c]))

    xT_all = np.zeros((NCORES, 128, ncols), NP_BF16)
    ap_all = np.zeros((NCORES, n_groups, 126, ncols), NP_BF16)
    inv_all = np.ones((NCORES, 1, ncols), np.float32)
    for c in range(NCORES):
        nodes = np.arange(c * NPC, (c + 1) * NPC)
        xT_all[c][:, col_of[nodes]] = x[nodes].T
        a = a_lists[c]
        inv_all[c, 0, col_of[a]] = 1.0 / cnt[a]

    e_core = src_s // NPC
    e_col = col_of[src_s]
    e_g = occ // SLOTS
    e_r = (occ % SLOTS) * 3
    for a in range(3):
        ap_all[e_core, e_g, e_r + a, e_col] = attr_s[:, a]
    inv_bf = inv_all.astype(NP_BF16)

    w1a = np.ascontiguousarray(W1[0:128])
    w1b = np.ascontiguousarray(W1[128:256])
    w1c_rep = np.ascontiguousarray(W1[256 + np.arange(126) % 3]).astype(NP_BF16)
    w2_bf = W2.astype(NP_BF16)
    b1c = np.ascontiguousarray(b1.reshape(128, 1))
    b2c = np.ascontiguousarray(b2.reshape(128, 1))
    ones = np.ones((1, 128), NP_BF16)

    return {
        "ncols": ncols, "n_groups": n_groups, "kinds": kinds,
        "xT_all": xT_all, "ap_all": ap_all, "inv_bf": inv_bf,
        "w1a": w1a, "w1b": w1b, "w1c_rep": w1c_rep, "w2_bf": w2_bf,
        "b1c": b1c, "b2c": b2c, "ones": ones, "col_of": col_of,
    }


def _in_maps(nm, p):
    maps = []
    for c in range(NCORES):
        m = {nm["xT"]: p["xT_all"][c], nm["inv"]: p["inv_bf"][c],
             nm["w1a"]: p["w1a"], nm["w1b"]: p["w1b"], nm["w1c"]: p["w1c_rep"],
             nm["w2"]: p["w2_bf"], nm["b1"]: p["b1c"], nm["b2"]: p["b2c"],
             nm["ones"]: p["ones"]}
        for g in range(p["n_groups"]):
            m[nm["ap"][g]] = p["ap_all"][c, g]
        maps.append(m)
    return maps


def _assemble(res, nm, col_of):
    out = np.empty((N, D), np.float32)
    for c in range(NCORES):
        outT = np.asarray(res.results[c][nm["out"]], np.float32)
        nodes = np.arange(c * NPC, (c + 1) * NPC)
        out[nodes] = outT[:, col_of[nodes]].T
    return out


def kernel(x, edge_index, edge_attr, u=None, batch=None, W1=None, b1=None,
           W2=None, b2=None, **_):
    p = _prepare(x, edge_index, edge_attr, W1, b1, W2, b2)
    nc, nm = _build(p["ncols"], p["n_groups"], p["kinds"])
    in_maps = _in_maps(nm, p)
    res = bass_utils.run_bass_kernel_spmd(nc, in_maps, core_ids=list(range(NCORES)))
    return _assemble(res, nm, p["col_of"])


# revision 8
# speedup vs baseline: 1.4371x; 1.4371x over previous
import sys

sys.path.insert(0, "/opt/trn_rl_repo")
import numpy as np
from concourse import bass, bacc, tile, bass_utils

mybir = bass.mybir
F32 = mybir.dt.float32
BF16 = mybir.dt.bfloat16
NP_BF16 = np.dtype(mybir.dt.np(BF16))

N = 100000
E = 1600000
D = 128
NCORES = 8
NPC = N // NCORES
CHUNK = 512
SLOTS = 42  # attr slots per matmul group (42*3 = 126 contraction rows)


def _build(ncols, n_groups, specs, reps=1):
    """specs: list of (kind, rows) per 512-col chunk; rows = 3*max_degree for
    'A' chunks (may exceed 126 -> multiple groups), 0 for 'B' chunks."""
    nc = bacc.Bacc(None, target_bir_lowering=False)
    xT_d = nc.dram_tensor("xT_d", [128, ncols], BF16, kind="ExternalInput")
    ap_d = nc.dram_tensor("ap_d", [126 * n_groups, ncols], BF16,
                          kind="ExternalInput")
    inv_d = nc.dram_tensor("inv_d", [1, ncols], BF16, kind="ExternalInput")
    w1a_d = nc.dram_tensor("w1a_d", [128, 128], F32, kind="ExternalInput")
    w1b_d = nc.dram_tensor("w1b_d", [128, 128], F32, kind="ExternalInput")
    w1c_d = nc.dram_tensor("w1c_d", [126, 128], BF16, kind="ExternalInput")
    w2_d = nc.dram_tensor("w2_d", [128, 128], BF16, kind="ExternalInput")
    b1_d = nc.dram_tensor("b1_d", [128, 1], F32, kind="ExternalInput")
    b2_d = nc.dram_tensor("b2_d", [128, 1], F32, kind="ExternalInput")
    ones_d = nc.dram_tensor("ones_d", [1, 128], BF16, kind="ExternalInput")
    out_d = nc.dram_tensor("out_d", [128, ncols], BF16, kind="ExternalOutput")

    has_b = any(k == "B" for k, _ in specs)
    relu = mybir.ActivationFunctionType.Relu
    ident = mybir.ActivationFunctionType.Identity

    with tile.TileContext(nc) as tc:
        with tc.tile_pool(name="const", bufs=1) as cp, \
             tc.tile_pool(name="work", bufs=3) as wp, \
             tc.tile_pool(name="ps", bufs=2, space="PSUM") as pp:
            inv = cp.tile([1, ncols], BF16, name="inv")
            w1a_f = cp.tile([128, 128], F32, name="w1a_f")
            w1b_f = cp.tile([128, 128], F32, name="w1b_f")
            w1ab_f = cp.tile([128, 128], F32, name="w1ab_f")
            w1ab = cp.tile([128, 128], BF16, name="w1ab")
            w1c = cp.tile([126, 128], BF16, name="w1c")
            w2 = cp.tile([128, 128], BF16, name="w2")
            b1 = cp.tile([128, 1], F32, name="b1")
            b2 = cp.tile([128, 1], F32, name="b2")
            ones = cp.tile([1, 128], BF16, name="ones")
            for t, dt_ in [(inv, inv_d), (w1a_f, w1a_d), (w1b_f, w1b_d),
                           (w1c, w1c_d), (w2, w2_d), (b1, b1_d), (b2, b2_d),
                           (ones, ones_d)]:
                nc.gpsimd.dma_start(t[:], dt_[:])

            nc.vector.tensor_tensor(out=w1ab_f[:], in0=w1a_f[:], in1=w1b_f[:],
                                    op=mybir.AluOpType.add)
            nc.vector.tensor_copy(w1ab[:], w1ab_f[:])
            if has_b:
                w1a_bf = cp.tile([128, 128], BF16, name="w1a_bf")
                nc.vector.tensor_copy(w1a_bf[:], w1a_f[:])

            for rep in range(reps):
              for i, (kind, rows) in enumerate(specs):
                lo, hi = i * CHUNK, (i + 1) * CHUNK
                xc = wp.tile([128, CHUNK], BF16, name="xc")
                nc.gpsimd.dma_start(xc[:], xT_d[:, lo:hi])
                P1 = pp.tile([128, CHUNK], F32, name="P1")
                if kind == "A":
                    Pinv = pp.tile([128, CHUNK], F32, name="Pinv")
                    nc.tensor.matmul(out=Pinv[:], lhsT=ones[:],
                                     rhs=inv[:, lo:hi], start=True, stop=True)
                    nc.tensor.matmul(out=P1[:], lhsT=w1ab[:], rhs=xc[:],
                                     start=True, stop=False)
                    n_g = -(-rows // 126)
                    for g in range(n_g):
                        rg = min(126, rows - 126 * g)
                        at = wp.tile([126, CHUNK], BF16, name=f"at{g}")
                        nc.gpsimd.dma_start(
                            at[:rg, :], ap_d[126 * g:126 * g + rg, lo:hi])
                        aps = wp.tile([126, CHUNK], BF16, name=f"aps{g}")
                        nc.vector.tensor_tensor(out=aps[:rg, :],
                                                in0=at[:rg, :],
                                                in1=Pinv[:rg, :],
                                                op=mybir.AluOpType.mult)
                        nc.tensor.matmul(out=P1[:], lhsT=w1c[:rg, :],
                                         rhs=aps[:rg, :], start=False,
                                         stop=(g == n_g - 1))
                else:
                    nc.tensor.matmul(out=P1[:], lhsT=w1a_bf[:], rhs=xc[:],
                                     start=True, stop=True)
                h = wp.tile([128, CHUNK], BF16, name="h")
                nc.scalar.activation(out=h[:], in_=P1[:], func=relu, bias=b1[:])
                Pout = pp.tile([128, CHUNK], F32, name="Pout")
                nc.tensor.matmul(out=Pout[:], lhsT=w2[:], rhs=h[:],
                                 start=True, stop=True)
                ostage = wp.tile([128, CHUNK], BF16, name="ostage")
                nc.scalar.activation(out=ostage[:], in_=Pout[:], func=ident,
                                     bias=b2[:])
                nc.gpsimd.dma_start(out_d[:, lo:hi], ostage[:])
    nc.compile()
    names = {
        "xT": xT_d.name, "ap": ap_d.name, "inv": inv_d.name,
        "w1a": w1a_d.name, "w1b": w1b_d.name, "w1c": w1c_d.name,
        "w2": w2_d.name, "b1": b1_d.name, "b2": b2_d.name,
        "ones": ones_d.name, "out": out_d.name,
    }
    return nc, names


def _prepare(x, edge_index, edge_attr, W1, b1, W2, b2):
    x = np.asarray(x, np.float32)
    attr = np.asarray(edge_attr, np.float32)
    src = np.asarray(edge_index)[1].astype(np.int64)
    W1 = np.asarray(W1, np.float32)
    b1 = np.asarray(b1, np.float32)
    W2 = np.asarray(W2, np.float32)
    b2 = np.asarray(b2, np.float32)

    cnt = np.bincount(src, minlength=N)
    order = np.argsort(src, kind="stable")
    src_s = src[order]
    attr_s = attr[order]
    rowptr = np.zeros(N + 1, np.int64)
    rowptr[1:] = np.cumsum(cnt)
    occ = np.arange(E, dtype=np.int64) - rowptr[src_s]
    maxdeg = int(cnt.max())
    n_groups = max(1, -(-maxdeg // SLOTS))

    # per-core columns: A-region (cnt>0, sorted by degree ascending) then
    # B-region (cnt==0); chunk ap rows = 3*max-degree over cores per chunk
    col_of = np.zeros(N, np.int64)
    a_lists, b_lists = [], []
    for c in range(NCORES):
        nodes = np.arange(c * NPC, (c + 1) * NPC)
        amask = cnt[nodes] > 0
        a = nodes[amask]
        a = a[np.argsort(cnt[a], kind="stable")]
        a_lists.append(a)
        b_lists.append(nodes[~amask])
    ca = max(-(-len(a) // CHUNK) for a in a_lists)
    cb = max(-(-len(b) // CHUNK) for b in b_lists)
    ncols = CHUNK * (ca + cb)
    for c in range(NCORES):
        col_of[a_lists[c]] = np.arange(len(a_lists[c]))
        col_of[b_lists[c]] = ca * CHUNK + np.arange(len(b_lists[c]))

    # per-chunk max degree across cores (A chunks)
    chunk_dmax = np.zeros(ca, np.int64)
    for c in range(NCORES):
        a = a_lists[c]
        deg = cnt[a]
        for i in range(ca):
            seg = deg[i * CHUNK:(i + 1) * CHUNK]
            if len(seg):
                chunk_dmax[i] = max(chunk_dmax[i], int(seg.max()))
    specs = [("A", int(3 * chunk_dmax[i])) for i in range(ca)]
    specs += [("B", 0)] * cb

    xT_all = np.zeros((NCORES, 128, ncols), NP_BF16)
    ap_all = np.zeros((NCORES, 126 * n_groups, ncols), NP_BF16)
    inv_all = np.ones((NCORES, 1, ncols), np.float32)
    for c in range(NCORES):
        nodes = np.arange(c * NPC, (c + 1) * NPC)
        xT_all[c][:, col_of[nodes]] = x[nodes].T
        a = a_lists[c]
        inv_all[c, 0, col_of[a]] = 1.0 / cnt[a]

    e_core = src_s // NPC
    e_col = col_of[src_s]
    e_row = (occ // SLOTS) * 126 + (occ % SLOTS) * 3
    for a in range(3):
        ap_all[e_core, e_row + a, e_col] = attr_s[:, a]
    inv_bf = inv_all.astype(NP_BF16)

    w1a = np.ascontiguousarray(W1[0:128])
    w1b = np.ascontiguousarray(W1[128:256])
    w1c_rep = np.ascontiguousarray(W1[256 + np.arange(126) % 3]).astype(NP_BF16)
    w2_bf = W2.astype(NP_BF16)
    b1c = np.ascontiguousarray(b1.reshape(128, 1))
    b2c = np.ascontiguousarray(b2.reshape(128, 1))
    ones = np.ones((1, 128), NP_BF16)

    return {
        "ncols": ncols, "n_groups": n_groups, "specs": specs,
        "xT_all": xT_all, "ap_all": ap_all, "inv_bf": inv_bf,
        "w1a": w1a, "w1b": w1b, "w1c_rep": w1c_rep, "w2_bf": w2_bf,
        "b1c": b1c, "b2c": b2c, "ones": ones, "col_of": col_of,
    }


def _in_maps(nm, p):
    maps = []
    for c in range(NCORES):
        m = {nm["xT"]: p["xT_all"][c], nm["ap"]: p["ap_all"][c],
             nm["inv"]: p["inv_bf"][c],
             nm["w1a"]: p["w1a"], nm["w1b"]: p["w1b"], nm["w1c"]: p["w1c_rep"],
             nm["w2"]: p["w2_bf"], nm["b1"]: p["b1c"], nm["b2"]: p["b2c"],
             nm["ones"]: p["ones"]}
        maps.append(m)
    return maps


def _assemble(res, nm, col_of):
    out = np.empty((N, D), np.float32)
    for c in range(NCORES):
        outT = np.asarray(res.results[c][nm["out"]]).astype(np.float32)
        nodes = np.arange(c * NPC, (c + 1) * NPC)
        out[nodes] = outT[:, col_of[nodes]].T
    return out


def kernel(x, edge_index, edge_attr, u=None, batch=None, W1=None, b1=None,
           W2=None, b2=None, **_):
    p = _prepare(x, edge_index, edge_attr, W1, b1, W2, b2)
    nc, nm = _build(p["ncols"], p["n_groups"], p["specs"])
    in_maps = _in_maps(nm, p)
    res = bass_utils.run_bass_kernel_spmd(nc, in_maps, core_ids=list(range(NCORES)))
    return _assemble(res, nm, p["col_of"])
